# revision 5
# baseline (speedup 1.0000x reference)
"""Trainium2 Bass kernel for the CIN (xDeepFM) block, v2.

inputs [2048,39,16] f32, W0 [1521,128], W1 [4992,128] -> out [2048,256] f32.
Data-parallel over the batch axis across 8 NeuronCores; weights replicated.

v2 over the 55.9us baseline:
- pair-sum build uses fp8e4 DoubleRow matmuls (selection matrix is exact in
  fp8; x carried as fp8 value+residual in the two k-subtiles) -> 0.5 cyc/col
- square evacuation split across ACT and DVE (GPSIMD has no PSUM port)
- out1 computed by an identity-matmul into the spare half of the out2 PSUM
  tile (start=False region trick) instead of a transpose + extra copy
- input tiles double-buffered (bufs=2) with a 2x-unrolled loop body so the
  next iteration's DMAs overlap compute; PE never idles at the boundary
"""


import ml_dtypes
import numpy as np

BF16 = ml_dtypes.bfloat16
F8 = ml_dtypes.float8_e4m3

B, M0, D = 2048, 39, 16
C0, C1 = 128, 128
NCORES = 8
BL = B // NCORES          # 256 batches per core
R = BL * D                # 4096 rows per core
NPAIR = (M0 * (M0 + 1)) // 2   # 780
KT = (NPAIR + M0 + 127) // 128 # 819 used rows -> 7 K-tiles
KPAD = KT * 128                # 896
RC = 512                       # r-chunk for the pair-sum build
NRC = R // RC                  # 8
NCHUNK = R // 128              # 32 chunks of (8 b x 16 d)
BPC = 128 // D                 # 8 batches per 128-row chunk
HW_ = BPC * (M0 + 1)           # 320
KP = 64                        # padded contraction rows for the DR matmuls


def host_constants(W0, W1):
    """Core-independent prepped tensors.

    Square trick: x_m*x_n = 0.5*(x_m+x_n)^2 - 0.5*x_m^2 - 0.5*x_n^2, so
      X1[r,:] = sum_p (0.5*W0sym[p])*sq_p[r] + sum_m Ah[m]*xsq[m,r]
    with sq_p = (x_m(p)+x_n(p))^2 built on the PE via the summed selection
    matrix (fp8 DoubleRow) and squared during the PSUM evacuation.
    """
    pairs = [(m, n) for m in range(M0) for n in range(m, M0)]
    assert len(pairs) == NPAIR

    selsum = np.zeros((128, KT, 128), dtype=np.float32)
    for p, (m, n) in enumerate(pairs):
        t, q = divmod(p, 128)
        selsum[m, t, q] += 1.0
        selsum[n, t, q] += 1.0
    for m in range(M0):
        t, q = divmod(NPAIR + m, 128)
        selsum[m, t, q] = 1.0

    # duplicate into both DoubleRow k-subtiles (j=0 pairs with x8, j=1 with
    # the fp8 residual of x)
    sel2 = np.zeros((KP, 2, KT, 128), dtype=np.float32)
    sel2[:M0, 0] = selsum[:M0]
    sel2[:M0, 1] = selsum[:M0]

    W0r = W0.reshape(M0, M0, C0)
    w0sym = np.zeros((KPAD, C0), dtype=np.float32)
    for p, (m, n) in enumerate(pairs):
        if m == n:
            w0sym[p] = W0r[m, m]
        else:
            w0sym[p] = W0r[m, n] + W0r[n, m]
    w0h_kt = 0.5 * w0sym.reshape(KT, 128, C0).transpose(1, 0, 2).copy()
    Ah = np.zeros((M0, C0), dtype=np.float32)
    for p, (m, n) in enumerate(pairs):
        Ah[m] -= 0.5 * w0sym[p]
        Ah[n] -= 0.5 * w0sym[p]
    for m in range(M0):
        t, q = divmod(NPAIR + m, 128)
        w0h_kt[q, t, :] = Ah[m]

    w1sb = W1.reshape(M0, C1, C0).transpose(1, 0, 2).copy()

    return dict(
        sel2=np.ascontiguousarray(sel2.astype(F8)),
        w0h=np.ascontiguousarray(w0h_kt.astype(BF16)),
        w1sb=np.ascontiguousarray(w1sb.astype(BF16)),
        ident=np.ascontiguousarray(np.eye(128, dtype=np.float32).astype(BF16)),
    )


def host_core_inputs(x_c):
    """Per-core prepped tensors from the [BL, M0, D] input shard."""
    xdT = np.zeros((M0, R), dtype=np.float32)
    xdT[:] = x_c.transpose(1, 0, 2).reshape(M0, R)
    x8 = xdT.astype(F8)
    xr8 = (xdT - x8.astype(np.float32)).astype(F8)
    xdT8i = np.zeros((KP, 2, R), dtype=F8)
    xdT8i[:M0, 0] = x8
    xdT8i[:M0, 1] = xr8

    # block-diagonal Gram operand with a ones column for the out1 d-sums
    xtt = x_c.reshape(NCHUNK, BPC, M0, D).transpose(1, 3, 0, 2)  # [b8,d,ch,m]
    xT2z = np.zeros((BPC, D, NCHUNK, BPC, M0 + 1), dtype=np.float32)
    for b8 in range(BPC):
        xT2z[b8, :, :, b8, :M0] = xtt[b8]
        xT2z[b8, :, :, b8, M0] = 1.0
    xT2z = xT2z.reshape(128, NCHUNK, HW_)
    return dict(
        x8i=np.ascontiguousarray(xdT8i),
        xt=np.ascontiguousarray(xT2z.astype(BF16)),
    )


def split_sync_waits(nc):
    """Rewrite every instruction carrying >1 sync wait: keep the first wait,
    hoist the rest onto same-engine NoOps inserted immediately before it."""
    import concourse.mybir as mybir

    counter = [0]
    for f in nc.m.functions:
        for bb in f.blocks:
            new_list = []
            changed = False
            for inst in bb.instructions:
                si = inst.sync_info
                waits = list(si.on_wait) if si is not None else []
                if len(waits) > 1:
                    changed = True
                    for w in waits[:-1]:
                        counter[0] += 1
                        nop = mybir.InstNoOp(
                            name=f"WSPLIT-{counter[0]}", ins=[], outs=[]
                        )
                        nop.engine = inst.engine
                        nop.sync_info = mybir.SyncInfo(on_wait=[w], on_update=[])
                        new_list.append(nop)
                    si.on_wait = waits[-1:]
                new_list.append(inst)
            if changed:
                bb.instructions = new_list
    return counter[0]


TP = [(0, 1), (2, 3), (4, 5), (6,)]


def build_program(reps=1, split_waits=True, loop_reps=None, cfg=None):
    """loop_reps: if set, wrap a 2x-unrolled body in a tc.For_i hardware loop
    with loop_reps//2 trips (so loop_reps = total kernel executions)."""
    import contextlib

    cfg = cfg or {}
    # "pool" = DVE copy PSUM->SBUF bf16, then GPSIMD square (no PSUM port
    # there, and walrus rejects two PSUM reads in one DVE tensor_tensor)
    SQ_ENG = cfg.get("sq_eng", ["act", "pool", "act", "act"])
    X1_ENG = cfg.get("x1_eng", ["act", "dve", "dve", "dve"])
    H_ENG = cfg.get("h_eng", ["act", "dve", "dve", "dve"])
    O_ENG = cfg.get("o_eng", "dve")

    import concourse.bass as bass
    import concourse.mybir as mybir
    import concourse.tile as tile

    f32 = mybir.dt.float32
    bf16 = mybir.dt.bfloat16
    fp8 = mybir.dt.float8e4
    DR = mybir.MatmulPerfMode.DoubleRow

    if loop_reps is not None:
        assert loop_reps % 2 == 0
        reps = 2
        trips = loop_reps // 2
    else:
        trips = None

    nc = bass.Bass("TRN2", target_bir_lowering=False, debug=False)
    d_x8i = nc.dram_tensor("x8i", [KP, 2, R], fp8, kind="ExternalInput")
    d_xt = nc.dram_tensor("xt", [128, NCHUNK, HW_], bf16, kind="ExternalInput")
    d_id = nc.dram_tensor("ident", [128, 128], bf16, kind="ExternalInput")
    d_sel = nc.dram_tensor("sel2", [KP, 2, KT, 128], fp8, kind="ExternalInput")
    d_w0 = nc.dram_tensor("w0h", [128, KT, C0], bf16, kind="ExternalInput")
    d_w1 = nc.dram_tensor("w1sb", [128, M0, C0], bf16, kind="ExternalInput")
    d_out = nc.dram_tensor("out", [BL, C0 + C1], f32, kind="ExternalOutput")

    def square_on(eng, out_ap, in_ap):
        if eng == "act":
            nc.scalar.square(out_ap, in_ap)
        else:
            nc.vector.tensor_mul(out_ap, in_ap, in_ap)

    def copy_on(eng, out_ap, in_ap):
        if eng == "act":
            nc.scalar.copy(out_ap, in_ap)
        else:
            nc.vector.tensor_copy(out_ap, in_ap)

    with tile.TileContext(nc) as tc:
        with (
            tc.tile_pool(name="const", bufs=2) as cpool,
            tc.tile_pool(name="x1sb", bufs=1) as x1pool,
            tc.tile_pool(name="hsb", bufs=1) as hpool,
            tc.tile_pool(name="sq", bufs=2) as sqpool,
            tc.tile_pool(name="stg", bufs=2) as stgpool,
            tc.tile_pool(name="outp", bufs=1) as opool,
            tc.tile_pool(name="ps_sp", bufs=2, space="PSUM") as ps_sp,
            tc.tile_pool(name="ps_x1", bufs=2, space="PSUM") as ps_x1,
            tc.tile_pool(name="ps_h", bufs=2, space="PSUM") as ps_h,
        ):
            loop_cm = (
                tc.For_i(
                    0,
                    trips,
                    1,
                    hint_engines=(
                        mybir.EngineType.PE,
                        mybir.EngineType.Activation,
                        mybir.EngineType.DVE,
                        mybir.EngineType.SP,
                    ),
                )
                if trips is not None
                else contextlib.nullcontext()
            )
            with loop_cm:
                for _rep in range(reps):
                    x8i = cpool.tile([KP, 2, R], fp8, tag="x8i")
                    sel2 = cpool.tile([KP, 2, KT, 128], fp8, tag="sel2")
                    w0 = cpool.tile([128, KT, C0], bf16, tag="w0")
                    w1 = cpool.tile([128, M0, C0], bf16, tag="w1")
                    xt = cpool.tile([128, NCHUNK, HW_], bf16, tag="xt")
                    ident = cpool.tile([128, 128], bf16, tag="ident")
                    nc.sync.dma_start(x8i[:, :, : R // 2], d_x8i[:, :, : R // 2])
                    nc.sync.dma_start(sel2[:], d_sel[:])
                    nc.sync.dma_start(w0[:], d_w0[:])
                    nc.sync.dma_start(
                        x8i[:, :, R // 2 :], d_x8i[:, :, R // 2 :]
                    )
                    for q in range(4):
                        cs = slice(q * (NCHUNK // 4), (q + 1) * (NCHUNK // 4))
                        nc.sync.dma_start(xt[:, cs, :], d_xt[:, cs, :])
                    nc.sync.dma_start(w1[:], d_w1[:])
                    nc.sync.dma_start(ident[:], d_id[:])

                    x1sb = x1pool.tile([128, NCHUNK, C0], bf16, tag="x1sb")
                    hsb = hpool.tile([128, NCHUNK, HW_], bf16, tag="hsb")
                    hsb3 = hsb[:].rearrange("n c (b m) -> n (c b) m", m=M0 + 1)
                    outsb = opool.tile([128, 2, C0 + C1], f32, tag="outsb")

                    def sel_group(rc, g, sqs):
                        grp = TP[g]
                        ng = len(grp)
                        rsl = slice(rc * RC, (rc + 1) * RC)
                        sum_ps = ps_sp.tile([128, 2, RC], f32, tag="sp")
                        for j, t in enumerate(grp):
                            nc.tensor.matmul(
                                sum_ps[:, j, :],
                                sel2[:, :, t, :],
                                x8i[:, :, rsl],
                                start=True,
                                stop=True,
                                perf_mode=DR,
                            )
                        sq = sqpool.tile([128, ng, RC], bf16, tag=f"sq{g}")
                        if SQ_ENG[g] == "pool":
                            stg = stgpool.tile(
                                [128, ng, RC], bf16, tag=f"st{g}"
                            )
                            nc.vector.tensor_copy(stg[:], sum_ps[:, :ng, :])
                            nc.gpsimd.tensor_mul(sq[:], stg[:], stg[:])
                        else:
                            square_on(SQ_ENG[g], sq[:], sum_ps[:, :ng, :])
                        for j, t in enumerate(grp):
                            sqs[t] = sq[:, j, :]

                    def emit_out(bt):
                        btsl = slice(bt * 128, (bt + 1) * 128)
                        o_ps = ps_x1.tile([128, 2 * C0], f32, tag="x1")
                        for m in range(M0):
                            nc.tensor.matmul(
                                o_ps[:, C0:],
                                hsb3[:, btsl, m],
                                w1[:, m, :],
                                start=(m == 0),
                                stop=(m == M0 - 1),
                            )
                        # out1 rides in the spare half of the bank: its bytes
                        # are pending-zero from the group start above, so the
                        # identity matmul accumulates onto zeros.
                        nc.tensor.matmul(
                            o_ps[:, :C0],
                            hsb3[:, btsl, M0],
                            ident[:],
                            start=False,
                            stop=True,
                            skip_group_check=True,
                        )
                        copy_on(O_ENG, outsb[:, bt, :], o_ps[:])
                        nc.sync.dma_start(d_out[btsl, :], outsb[:, bt, :])

                    cur_sqs = [None] * KT
                    for g in range(4):
                        sel_group(0, g, cur_sqs)
                    nxt_sqs = [None] * KT
                    for rc in range(NRC):
                        for rs in range(RC // 128):
                            ch = rc * (RC // 128) + rs
                            csl = slice(rs * 128, (rs + 1) * 128)
                            x1t = ps_x1.tile([128, 2 * C0], f32, tag="x1")
                            for t in range(KT):
                                nc.tensor.matmul(
                                    x1t[:, :C0],
                                    cur_sqs[t][:, csl],
                                    w0[:, t, :],
                                    start=(t == 0),
                                    stop=(t == KT - 1),
                                )
                            copy_on(X1_ENG[rs], x1sb[:, ch, :], x1t[:, :C0])
                            h_ps = ps_h.tile([128, HW_], f32, tag="h")
                            nc.tensor.matmul(
                                h_ps[:],
                                x1sb[:, ch, :],
                                xt[:, ch, :],
                                start=True,
                                stop=True,
                            )
                            copy_on(H_ENG[rs], hsb[:, ch, :], h_ps[:])
                            if rc < NRC - 1:
                                sel_group(rc + 1, rs, nxt_sqs)
                            if ch == 15:
                                emit_out(0)
                            elif ch == 31:
                                emit_out(1)
                        if rc < NRC - 1:
                            cur_sqs, nxt_sqs = nxt_sqs, [None] * KT

    if split_waits:
        split_sync_waits(nc)
    return nc


def make_in_maps(inputs, W0, W1):
    consts = host_constants(np.asarray(W0), np.asarray(W1))
    in_maps = []
    for c in range(NCORES):
        x_c = np.ascontiguousarray(np.asarray(inputs)[c * BL : (c + 1) * BL])
        m = dict(consts)
        m.update(host_core_inputs(x_c))
        in_maps.append(m)
    return in_maps


_KERNEL_CACHE = {}


def kernel(inputs, W0, W1):
    inputs = np.ascontiguousarray(np.asarray(inputs, dtype=np.float32))
    W0 = np.ascontiguousarray(np.asarray(W0, dtype=np.float32))
    W1 = np.ascontiguousarray(np.asarray(W1, dtype=np.float32))
    assert inputs.shape == (B, M0, D) and W0.shape == (M0 * M0, C0)
    assert W1.shape == (M0 * C0, C1)

    if "nc" not in _KERNEL_CACHE:
        _KERNEL_CACHE["nc"] = build_program()
    nc = _KERNEL_CACHE["nc"]

    in_maps = make_in_maps(inputs, W0, W1)

    from concourse.bass_utils import run_bass_kernel_spmd

    res = run_bass_kernel_spmd(nc, in_maps, core_ids=list(range(NCORES)))
    out = np.concatenate([res.results[c]["out"] for c in range(NCORES)], axis=0)
    return np.ascontiguousarray(out.astype(np.float32))
